# revision 4
# baseline (speedup 1.0000x reference)
"""HVAE loss kernel for Trainium2 (8 NeuronCores, SPMD row-sharded).

Math: BCEWithLogits(x, adj) * N^2 = sum(softplus(x)) - sum(x at edge positions),
with softplus(x) = x + ln(1 + exp(-x)) (safe for |x| <~ 10, inputs are randn).
The adjacency is never materialized: host packs a N x N/8 bitmask (exact set
semantics); the device extracts each bit plane with bitwise_and against a u8
scalar AP, then (mask * 2^-b) * x is summed by a fused multiply-accumulate.
Engines: DMA streams 33 MB/core; ACT does exp + ln1p (+ KL square/exp); DVE does
the 15 mod-mult-accumulate ops; PE accumulates sum(x) via a ones-vector matmul.
"""

import numpy as np

N = 8192
D = 64
NCORES = 8
RPC = N // NCORES          # rows per core: 1024
NBLK = RPC // 128          # 128-row blocks per core: 8
NBYTES = N // 8            # bitmask bytes per row: 1024
W = N                      # row width
LOG2PI = float(np.log(2.0 * np.pi))
Q_LOGVAR = float(np.log(0.25))

_compiled = None


def _build_nc():
    import concourse.bass as bass
    import concourse.mybir as mybir

    AF = mybir.ActivationFunctionType
    ALU = mybir.AluOpType
    f32 = mybir.dt.float32
    u8 = mybir.dt.uint8

    nc = bass.Bass()
    x_in = nc.declare_dram_parameter("xs", [RPC, W], f32, isOutput=False)
    b_in = nc.declare_dram_parameter("bits", [RPC, NBYTES], u8, isOutput=False)
    zmue_in = nc.declare_dram_parameter("zmue_t", [D, RPC], f32, isOutput=False)
    zlve_in = nc.declare_dram_parameter("zlve_t", [D, RPC], f32, isOutput=False)
    zmun_in = nc.declare_dram_parameter("zmun_t", [D, RPC], f32, isOutput=False)
    zlvn_in = nc.declare_dram_parameter("zlvn_t", [D, RPC], f32, isOutput=False)
    negA_in = nc.declare_dram_parameter("negA", [D, 1], f32, isOutput=False)
    negB_in = nc.declare_dram_parameter("negB", [D, 1], f32, isOutput=False)
    c_in = nc.declare_dram_parameter("consts", [128, 2], f32, isOutput=False)
    m_in = nc.declare_dram_parameter("mconst", [128, 8], u8, isOutput=False)
    sp_out = nc.declare_dram_parameter("sp_acc", [128, NBLK], f32, isOutput=True)
    bit_out = nc.declare_dram_parameter("bit_acc", [128, 8 * NBLK], f32, isOutput=True)
    xs_out = nc.declare_dram_parameter("xs_sb", [1, 512], f32, isOutput=True)
    kl_out = nc.declare_dram_parameter("kl_acc", [D, 6], f32, isOutput=True)

    from contextlib import ExitStack
    with ExitStack() as es:
        sb = lambda name, shape, dt: es.enter_context(nc.sbuf_tensor(name, shape, dt))
        sem = lambda name: es.enter_context(nc.semaphore(name))
        xb0 = sb("xb0", [128, W], f32)
        xb1 = sb("xb1", [128, W], f32)
        xb2 = sb("xb2", [128, W], f32)
        bb0 = sb("bb0", [128, NBYTES], u8)
        bb1 = sb("bb1", [128, NBYTES], u8)
        et = sb("et", [128, W], f32)
        mc = sb("mc", [128, 8], u8)
        mask8 = sb("mask8", [128, NBYTES], u8)
        scr = sb("scr", [128, NBYTES], f32)
        consts = sb("consts_sb", [128, 2], f32)
        spa = sb("spa", [128, NBLK], f32)
        bita = sb("bita", [128, 8 * NBLK], f32)
        xssb = sb("xssb", [1, 512], f32)
        zmue = sb("zmue_sb", [D, RPC], f32)
        zlve = sb("zlve_sb", [D, RPC], f32)
        zmun = sb("zmun_sb", [D, RPC], f32)
        zlvn = sb("zlvn_sb", [D, RPC], f32)
        negA = sb("negA_sb", [D, 1], f32)
        negB = sb("negB_sb", [D, 1], f32)
        kscr = sb("kscr", [D, RPC], f32)
        kla = sb("kla", [D, 6], f32)
        xsem0, xsem1, xsem2 = sem("xsem0"), sem("xsem1"), sem("xsem2")
        bsem0, bsem1 = sem("bsem0"), sem("bsem1")
        csem, zsem = sem("csem"), sem("zsem")
        act_sem, dve_sem, pe_sem, osem = (sem("act_sem"), sem("dve_sem"),
                                          sem("pe_sem"), sem("osem"))
        xbufs = [xb0, xb1, xb2]
        bbufs = [bb0, bb1]
        xsems = [xsem0, xsem1, xsem2]
        bsems = [bsem0, bsem1]
        ps = nc.alloc_psum_tensor("xs_ps", [1, 512], f32)

        with nc.Block() as block:

            @block.sync
            def _(sync):
                sync.dma_start(out=consts[:, :], in_=c_in[:, :]).then_inc(csem, 16)
                sync.dma_start(out=mc[:, :], in_=m_in[:, :]).then_inc(csem, 16)
                for g in range(NBLK):
                    if g >= 3:
                        sync.wait_ge(act_sem, g - 2)
                        sync.wait_ge(dve_sem, g - 2)
                        sync.wait_ge(pe_sem, g - 2)
                    sync.dma_start(
                        out=xbufs[g % 3][:, :], in_=x_in[128 * g:128 * (g + 1), :]
                    ).then_inc(xsems[g % 3], 16)
                    if g >= 2:
                        sync.wait_ge(dve_sem, g - 1)
                    sync.dma_start(
                        out=bbufs[g % 2][:, :], in_=b_in[128 * g:128 * (g + 1), :]
                    ).then_inc(bsems[g % 2], 16)
                for src, dst in ((zmue_in, zmue), (zlve_in, zlve), (zmun_in, zmun),
                                 (zlvn_in, zlvn), (negA_in, negA), (negB_in, negB)):
                    sync.dma_start(out=dst[:, :], in_=src[:, :]).then_inc(zsem, 16)
                sync.wait_ge(act_sem, NBLK + 1)
                sync.wait_ge(dve_sem, NBLK + 3)
                sync.dma_start(out=sp_out[:, :], in_=spa[:, :]).then_inc(osem, 16)
                sync.dma_start(out=bit_out[:, :], in_=bita[:, :]).then_inc(osem, 16)
                sync.dma_start(out=xs_out[:, :], in_=xssb[:, :]).then_inc(osem, 16)
                sync.dma_start(out=kl_out[:, :], in_=kla[:, :]).then_inc(osem, 16)
                sync.wait_ge(osem, 64)

            @block.scalar
            def _(scalar):
                scalar.wait_ge(csem, 32)
                zeros128 = consts[:, 0:1]
                ones128 = consts[:, 1:2]
                for g in range(NBLK):
                    scalar.wait_ge(xsems[g % 3], 16 * (g // 3 + 1))
                    scalar.activation(et[:, :], xbufs[g % 3][:, :], AF.Exp,
                                      bias=zeros128, scale=-1.0)
                    scalar.activation(et[:, :], et[:, :], AF.Ln, bias=ones128,
                                      accum_out=spa[:, g:g + 1]).then_inc(act_sem, 1)
                scalar.wait_ge(zsem, 96)
                zeros64 = consts[0:D, 0:1]
                scalar.activation(kscr[:, :], zmue[:, :], AF.Square,
                                  bias=negA[:, :], accum_out=kla[:, 0:1])
                scalar.activation(kscr[:, :], zlve[:, :], AF.Exp,
                                  bias=zeros64, accum_out=kla[:, 1:2])
                scalar.activation(kscr[:, :], zmun[:, :], AF.Square,
                                  bias=negB[:, :], accum_out=kla[:, 3:4])
                scalar.activation(kscr[:, :], zlvn[:, :], AF.Exp,
                                  bias=zeros64, accum_out=kla[:, 4:5]).then_inc(act_sem, 1)

            @block.vector
            def _(vector):
                vector.wait_ge(csem, 32)
                for g in range(NBLK):
                    vector.wait_ge(xsems[g % 3], 16 * (g // 3 + 1))
                    vector.wait_ge(bsems[g % 2], 16 * (g // 2 + 1))
                    x = xbufs[g % 3]
                    bb = bbufs[g % 2]
                    last = None
                    for b in range(8):
                        vector.tensor_scalar(out=mask8[:, :], in0=bb[:, :],
                                             scalar1=mc[:, b:b + 1], scalar2=None,
                                             op0=ALU.bitwise_and)
                        last = vector.scalar_tensor_tensor(
                            out=scr[:, :], in0=mask8[:, :], scalar=float(2.0 ** -b),
                            in1=x[:, b * NBYTES:(b + 1) * NBYTES],
                            op0=ALU.mult, op1=ALU.mult,
                            accum_out=bita[:, 8 * g + b:8 * g + b + 1])
                    last.then_inc(dve_sem, 1)
                vector.wait_ge(zsem, 96)
                vector.tensor_reduce(out=kla[:, 2:3], in_=zlve[:, :],
                                     axis=mybir.AxisListType.X,
                                     op=ALU.add)
                vector.tensor_reduce(out=kla[:, 5:6], in_=zlvn[:, :],
                                     axis=mybir.AxisListType.X,
                                     op=ALU.add).then_inc(dve_sem, 1)
                vector.wait_ge(pe_sem, NBLK)
                vector.tensor_copy(xssb[:, :], ps[:, :]).then_inc(dve_sem, 2)

            @block.tensor
            def _(tensor):
                tensor.wait_ge(csem, 32)
                ones128 = consts[:, 1:2]
                for g in range(NBLK):
                    tensor.wait_ge(xsems[g % 3], 16 * (g // 3 + 1))
                    last = None
                    for s in range(W // 512):
                        last = tensor.matmul(
                            ps[:, :], ones128, xbufs[g % 3][:, 512 * s:512 * (s + 1)],
                            start=(g == 0 and s == 0),
                            stop=(g == NBLK - 1 and s == W // 512 - 1))
                    last.then_inc(pe_sem, 1)

    import concourse.mybir as mybir  # noqa: F811  (kept local for clarity)
    return nc


def _host_prep(edge_logits, edge_index, z_mu_n, z_logvar_n, z_mu_e, z_logvar_e,
               mu_Alpha, mu_Beta):
    i = np.asarray(edge_index[0], dtype=np.int64)
    j = np.asarray(edge_index[1], dtype=np.int64)
    adj = np.zeros((N, N), dtype=bool)
    adj[i, j] = True
    adj[j, i] = True
    packed = np.packbits(
        adj.reshape(N, 8, NBYTES).transpose(0, 2, 1), axis=2, bitorder="little"
    ).reshape(N, NBYTES)

    consts = np.zeros((128, 2), np.float32)
    consts[:, 1] = 1.0
    mconst = np.broadcast_to(np.array([1 << b for b in range(8)], np.uint8),
                             (128, 8)).copy()
    zT = [np.ascontiguousarray(np.asarray(t, np.float32).T)
          for t in (z_mu_e, z_logvar_e, z_mu_n, z_logvar_n)]
    negA = np.ascontiguousarray(-np.asarray(mu_Alpha, np.float32).reshape(D, 1))
    negB = np.ascontiguousarray(-np.asarray(mu_Beta, np.float32).reshape(D, 1))

    x = np.ascontiguousarray(np.asarray(edge_logits, np.float32))
    in_maps = []
    for c in range(NCORES):
        r0, r1 = RPC * c, RPC * (c + 1)
        in_maps.append({
            "xs": x[r0:r1],
            "bits": packed[r0:r1],
            "zmue_t": np.ascontiguousarray(zT[0][:, r0:r1]),
            "zlve_t": np.ascontiguousarray(zT[1][:, r0:r1]),
            "zmun_t": np.ascontiguousarray(zT[2][:, r0:r1]),
            "zlvn_t": np.ascontiguousarray(zT[3][:, r0:r1]),
            "negA": negA,
            "negB": negB,
            "consts": consts,
            "mconst": mconst,
        })
    return in_maps


def kernel(z_mu_n, z_logvar_n, z_mu_e, z_logvar_e, Alpha_mu, Beta_mu,
           edge_logits, mu_Alpha, mu_Beta, edge_index, num_nodes):
    global _compiled
    from concourse.bass_utils import run_bass_kernel_spmd

    if _compiled is None:
        _compiled = _build_nc()
    in_maps = _host_prep(edge_logits, edge_index, z_mu_n, z_logvar_n,
                         z_mu_e, z_logvar_e, mu_Alpha, mu_Beta)
    res = run_bass_kernel_spmd(_compiled, in_maps, list(range(NCORES)))
    return _combine(res.results, Alpha_mu, Beta_mu, mu_Alpha, mu_Beta)


def _combine(results, Alpha_mu, Beta_mu, mu_Alpha, mu_Beta):
    sp_ln = 0.0   # sum ln(1+exp(-x))
    x_sum = 0.0   # sum x
    edge_sum = 0.0  # sum of x at edge positions
    kl = np.zeros(6, dtype=np.float64)
    for r in results:
        sp_ln += r["sp_acc"].astype(np.float64).sum()
        x_sum += r["xs_sb"].astype(np.float64).sum()
        edge_sum += r["bit_acc"].astype(np.float64).sum()
        kl += r["kl_acc"].astype(np.float64).sum(axis=0)

    n2 = float(N) * float(N)
    logpx_z = (x_sum + sp_ln - edge_sum) / n2

    nd = float(N) * float(D)
    sq_e, exp_e, lv_e, sq_n, exp_n, lv_n = kl
    kl_structure = -0.5 * ((1.0 - Q_LOGVAR) * nd + lv_e - 4.0 * (sq_e + exp_e)) / nd
    kl_semantic = -0.5 * ((1.0 - Q_LOGVAR) * nd + lv_n - 4.0 * (sq_n + exp_n)) / nd

    mu_A = np.asarray(mu_Alpha, np.float64)
    mu_B = np.asarray(mu_Beta, np.float64)
    A_mu = np.asarray(Alpha_mu, np.float64)
    B_mu = np.asarray(Beta_mu, np.float64)
    log_pmu_Alpha = float(np.mean(-0.5 * (LOG2PI + mu_A ** 2)))
    log_pmu_Beta = float(np.mean(-0.5 * (LOG2PI + mu_B ** 2)))
    extra_kl_Alpha = float(np.mean(2.0 * (mu_A - A_mu) ** 2))
    extra_kl_Beta = float(np.mean(2.0 * (mu_B - B_mu) ** 2))

    total = (log_pmu_Alpha + extra_kl_Alpha + log_pmu_Beta + extra_kl_Beta
             + logpx_z + kl_structure + kl_semantic)
    return np.float32(total)


import concourse.mybir as mybir  # noqa: E402


# revision 5
# speedup vs baseline: 93614.8692x; 93614.8692x over previous
"""HVAE loss kernel for Trainium2 (8 NeuronCores, SPMD row-sharded).

Math: BCEWithLogits(x, adj) * N^2 = sum(softplus(x)) - sum(x at edge positions),
with softplus(x) = x + ln(1 + exp(-x)) (safe for |x| <~ 10, inputs are randn).
The adjacency is never materialized: host packs a N x N/8 bitmask (exact set
semantics); the device extracts each bit plane with bitwise_and against a u8
scalar AP, then (mask * 2^-b) * x is summed by a fused multiply-accumulate.
Engines: DMA streams 33 MB/core; ACT does exp + ln1p (+ KL square/exp); DVE does
the 15 mod-mult-accumulate ops; PE accumulates sum(x) via a ones-vector matmul.
"""

import numpy as np

N = 8192
D = 64
NCORES = 8
RPC = N // NCORES          # rows per core: 1024
NBLK = RPC // 128          # 128-row blocks per core: 8
NBYTES = N // 8            # bitmask bytes per row: 1024
W = N                      # row width
LOG2PI = float(np.log(2.0 * np.pi))
Q_LOGVAR = float(np.log(0.25))

_compiled = None


def _build_nc(reps=1):
    import concourse.bass as bass
    import concourse.mybir as mybir

    AF = mybir.ActivationFunctionType
    ALU = mybir.AluOpType
    f32 = mybir.dt.float32
    u8 = mybir.dt.uint8

    nc = bass.Bass()
    x_in = nc.declare_dram_parameter("xs", [RPC, W], f32, isOutput=False)
    b_in = nc.declare_dram_parameter("bits", [RPC, NBYTES], u8, isOutput=False)
    zmue_in = nc.declare_dram_parameter("zmue_t", [D, RPC], f32, isOutput=False)
    zlve_in = nc.declare_dram_parameter("zlve_t", [D, RPC], f32, isOutput=False)
    zmun_in = nc.declare_dram_parameter("zmun_t", [D, RPC], f32, isOutput=False)
    zlvn_in = nc.declare_dram_parameter("zlvn_t", [D, RPC], f32, isOutput=False)
    negA_in = nc.declare_dram_parameter("negA", [D, 1], f32, isOutput=False)
    negB_in = nc.declare_dram_parameter("negB", [D, 1], f32, isOutput=False)
    c_in = nc.declare_dram_parameter("consts", [128, 2], f32, isOutput=False)
    m_in = nc.declare_dram_parameter("mconst", [128, 8], u8, isOutput=False)
    sp_out = nc.declare_dram_parameter("sp_acc", [128, NBLK], f32, isOutput=True)
    bit_out = nc.declare_dram_parameter("bit_acc", [128, 8 * NBLK], f32, isOutput=True)
    xs_out = nc.declare_dram_parameter("xs_sb", [1, 512], f32, isOutput=True)
    kl_out = nc.declare_dram_parameter("kl_acc", [D, 6], f32, isOutput=True)

    from contextlib import ExitStack
    with ExitStack() as es:
        sb = lambda name, shape, dt: es.enter_context(nc.sbuf_tensor(name, shape, dt))
        sem = lambda name: es.enter_context(nc.semaphore(name))
        xb0 = sb("xb0", [128, W], f32)
        xb1 = sb("xb1", [128, W], f32)
        xb2 = sb("xb2", [128, W], f32)
        bb0 = sb("bb0", [128, NBYTES], u8)
        bb1 = sb("bb1", [128, NBYTES], u8)
        et = sb("et", [128, W], f32)
        mc = sb("mc", [128, 8], u8)
        mask8 = sb("mask8", [128, NBYTES], u8)
        scr = sb("scr", [128, NBYTES], f32)
        consts = sb("consts_sb", [128, 2], f32)
        spa = sb("spa", [128, NBLK], f32)
        bita = sb("bita", [128, 8 * NBLK], f32)
        xssb = sb("xssb", [1, 512], f32)
        zmue = sb("zmue_sb", [D, RPC], f32)
        zlve = sb("zlve_sb", [D, RPC], f32)
        zmun = sb("zmun_sb", [D, RPC], f32)
        zlvn = sb("zlvn_sb", [D, RPC], f32)
        negA = sb("negA_sb", [D, 1], f32)
        negB = sb("negB_sb", [D, 1], f32)
        kscr = sb("kscr", [D, RPC], f32)
        kla = sb("kla", [D, 6], f32)
        xsem0, xsem1, xsem2 = sem("xsem0"), sem("xsem1"), sem("xsem2")
        bsem0, bsem1 = sem("bsem0"), sem("bsem1")
        csem, zsem = sem("csem"), sem("zsem")
        act_sem, dve_sem, pe_sem, osem = (sem("act_sem"), sem("dve_sem"),
                                          sem("pe_sem"), sem("osem"))
        xbufs = [xb0, xb1, xb2]
        bbufs = [bb0, bb1]
        xsems = [xsem0, xsem1, xsem2]
        bsems = [bsem0, bsem1]
        ps = nc.alloc_psum_tensor("xs_ps", [1, 512], f32)

        with nc.Block() as block:

            @block.sync
            def _(sync):
                sync.dma_start(out=consts[:, :], in_=c_in[:, :]).then_inc(csem, 16)
                sync.dma_start(out=mc[:, :], in_=m_in[:, :]).then_inc(csem, 16)
                for g in range(reps * NBLK):
                    gg = g % NBLK
                    if g >= 3:
                        sync.wait_ge(act_sem, g - 2)
                        sync.wait_ge(dve_sem, g - 2)
                        sync.wait_ge(pe_sem, g - 2)
                    sync.dma_start(
                        out=xbufs[g % 3][:, :], in_=x_in[128 * gg:128 * (gg + 1), :]
                    ).then_inc(xsems[g % 3], 16)
                    if g >= 2:
                        sync.wait_ge(dve_sem, g - 1)
                    sync.dma_start(
                        out=bbufs[g % 2][:, :], in_=b_in[128 * gg:128 * (gg + 1), :]
                    ).then_inc(bsems[g % 2], 16)
                for src, dst in ((zmue_in, zmue), (zlve_in, zlve), (zmun_in, zmun),
                                 (zlvn_in, zlvn), (negA_in, negA), (negB_in, negB)):
                    sync.dma_start(out=dst[:, :], in_=src[:, :]).then_inc(zsem, 16)
                sync.wait_ge(act_sem, reps * NBLK + 1)
                sync.wait_ge(dve_sem, reps * NBLK + 3)
                sync.dma_start(out=sp_out[:, :], in_=spa[:, :]).then_inc(osem, 16)
                sync.dma_start(out=bit_out[:, :], in_=bita[:, :]).then_inc(osem, 16)
                sync.dma_start(out=xs_out[:, :], in_=xssb[:, :]).then_inc(osem, 16)
                sync.dma_start(out=kl_out[:, :], in_=kla[:, :]).then_inc(osem, 16)
                sync.wait_ge(osem, 64)

            @block.scalar
            def _(scalar):
                scalar.wait_ge(csem, 32)
                zeros128 = consts[:, 0:1]
                ones128 = consts[:, 1:2]
                for g in range(reps * NBLK):
                    scalar.wait_ge(xsems[g % 3], 16 * (g // 3 + 1))
                    scalar.activation(et[:, :], xbufs[g % 3][:, :], AF.Exp,
                                      bias=zeros128, scale=-1.0)
                    scalar.activation(et[:, :], et[:, :], AF.Ln, bias=ones128,
                                      accum_out=spa[:, g % NBLK:g % NBLK + 1]).then_inc(act_sem, 1)
                scalar.wait_ge(zsem, 96)
                zeros64 = consts[0:D, 0:1]
                scalar.activation(kscr[:, :], zmue[:, :], AF.Square,
                                  bias=negA[:, :], accum_out=kla[:, 0:1])
                scalar.activation(kscr[:, :], zlve[:, :], AF.Exp,
                                  bias=zeros64, accum_out=kla[:, 1:2])
                scalar.activation(kscr[:, :], zmun[:, :], AF.Square,
                                  bias=negB[:, :], accum_out=kla[:, 3:4])
                scalar.activation(kscr[:, :], zlvn[:, :], AF.Exp,
                                  bias=zeros64, accum_out=kla[:, 4:5]).then_inc(act_sem, 1)

            @block.vector
            def _(vector):
                vector.wait_ge(csem, 32)
                for g in range(reps * NBLK):
                    gg = g % NBLK
                    vector.wait_ge(xsems[g % 3], 16 * (g // 3 + 1))
                    vector.wait_ge(bsems[g % 2], 16 * (g // 2 + 1))
                    x = xbufs[g % 3]
                    bb = bbufs[g % 2]
                    last = None
                    for b in range(8):
                        vector.tensor_scalar(out=mask8[:, :], in0=bb[:, :],
                                             scalar1=mc[:, b:b + 1], scalar2=None,
                                             op0=ALU.bitwise_and)
                        last = vector.scalar_tensor_tensor(
                            out=scr[:, :], in0=mask8[:, :], scalar=float(2.0 ** -b),
                            in1=x[:, b * NBYTES:(b + 1) * NBYTES],
                            op0=ALU.mult, op1=ALU.mult,
                            accum_out=bita[:, 8 * gg + b:8 * gg + b + 1])
                    last.then_inc(dve_sem, 1)
                vector.wait_ge(zsem, 96)
                vector.tensor_reduce(out=kla[:, 2:3], in_=zlve[:, :],
                                     axis=mybir.AxisListType.X,
                                     op=ALU.add)
                vector.tensor_reduce(out=kla[:, 5:6], in_=zlvn[:, :],
                                     axis=mybir.AxisListType.X,
                                     op=ALU.add).then_inc(dve_sem, 1)
                vector.wait_ge(pe_sem, reps * NBLK)
                vector.tensor_copy(xssb[:, :], ps[:, :]).then_inc(dve_sem, 2)

            @block.tensor
            def _(tensor):
                tensor.wait_ge(csem, 32)
                ones128 = consts[:, 1:2]
                for g in range(reps * NBLK):
                    tensor.wait_ge(xsems[g % 3], 16 * (g // 3 + 1))
                    last = None
                    for s in range(W // 512):
                        last = tensor.matmul(
                            ps[:, :], ones128, xbufs[g % 3][:, 512 * s:512 * (s + 1)],
                            start=(g == 0 and s == 0),
                            stop=(g == reps * NBLK - 1 and s == W // 512 - 1))
                    last.then_inc(pe_sem, 1)

    import concourse.mybir as mybir  # noqa: F811  (kept local for clarity)
    return nc


def _host_prep(edge_logits, edge_index, z_mu_n, z_logvar_n, z_mu_e, z_logvar_e,
               mu_Alpha, mu_Beta):
    i = np.asarray(edge_index[0], dtype=np.int64)
    j = np.asarray(edge_index[1], dtype=np.int64)
    adj = np.zeros((N, N), dtype=bool)
    adj[i, j] = True
    adj[j, i] = True
    packed = np.packbits(
        adj.reshape(N, 8, NBYTES).transpose(0, 2, 1), axis=2, bitorder="little"
    ).reshape(N, NBYTES)

    consts = np.zeros((128, 2), np.float32)
    consts[:, 1] = 1.0
    mconst = np.broadcast_to(np.array([1 << b for b in range(8)], np.uint8),
                             (128, 8)).copy()
    zT = [np.ascontiguousarray(np.asarray(t, np.float32).T)
          for t in (z_mu_e, z_logvar_e, z_mu_n, z_logvar_n)]
    negA = np.ascontiguousarray(-np.asarray(mu_Alpha, np.float32).reshape(D, 1))
    negB = np.ascontiguousarray(-np.asarray(mu_Beta, np.float32).reshape(D, 1))

    x = np.ascontiguousarray(np.asarray(edge_logits, np.float32))
    in_maps = []
    for c in range(NCORES):
        r0, r1 = RPC * c, RPC * (c + 1)
        in_maps.append({
            "xs": x[r0:r1],
            "bits": packed[r0:r1],
            "zmue_t": np.ascontiguousarray(zT[0][:, r0:r1]),
            "zlve_t": np.ascontiguousarray(zT[1][:, r0:r1]),
            "zmun_t": np.ascontiguousarray(zT[2][:, r0:r1]),
            "zlvn_t": np.ascontiguousarray(zT[3][:, r0:r1]),
            "negA": negA,
            "negB": negB,
            "consts": consts,
            "mconst": mconst,
        })
    return in_maps


def kernel(z_mu_n, z_logvar_n, z_mu_e, z_logvar_e, Alpha_mu, Beta_mu,
           edge_logits, mu_Alpha, mu_Beta, edge_index, num_nodes):
    global _compiled
    from concourse.bass_utils import run_bass_kernel_spmd

    if _compiled is None:
        _compiled = _build_nc()
    in_maps = _host_prep(edge_logits, edge_index, z_mu_n, z_logvar_n,
                         z_mu_e, z_logvar_e, mu_Alpha, mu_Beta)
    res = run_bass_kernel_spmd(_compiled, in_maps, list(range(NCORES)))
    return _combine(res.results, Alpha_mu, Beta_mu, mu_Alpha, mu_Beta)


def _combine(results, Alpha_mu, Beta_mu, mu_Alpha, mu_Beta):
    sp_ln = 0.0   # sum ln(1+exp(-x))
    x_sum = 0.0   # sum x
    edge_sum = 0.0  # sum of x at edge positions
    kl = np.zeros(6, dtype=np.float64)
    for r in results:
        sp_ln += r["sp_acc"].astype(np.float64).sum()
        x_sum += r["xs_sb"].astype(np.float64).sum()
        edge_sum += r["bit_acc"].astype(np.float64).sum()
        kl += r["kl_acc"].astype(np.float64).sum(axis=0)

    n2 = float(N) * float(N)
    logpx_z = (x_sum + sp_ln - edge_sum) / n2

    nd = float(N) * float(D)
    sq_e, exp_e, lv_e, sq_n, exp_n, lv_n = kl
    kl_structure = -0.5 * ((1.0 - Q_LOGVAR) * nd + lv_e - 4.0 * (sq_e + exp_e)) / nd
    kl_semantic = -0.5 * ((1.0 - Q_LOGVAR) * nd + lv_n - 4.0 * (sq_n + exp_n)) / nd

    mu_A = np.asarray(mu_Alpha, np.float64)
    mu_B = np.asarray(mu_Beta, np.float64)
    A_mu = np.asarray(Alpha_mu, np.float64)
    B_mu = np.asarray(Beta_mu, np.float64)
    log_pmu_Alpha = float(np.mean(-0.5 * (LOG2PI + mu_A ** 2)))
    log_pmu_Beta = float(np.mean(-0.5 * (LOG2PI + mu_B ** 2)))
    extra_kl_Alpha = float(np.mean(2.0 * (mu_A - A_mu) ** 2))
    extra_kl_Beta = float(np.mean(2.0 * (mu_B - B_mu) ** 2))

    total = (log_pmu_Alpha + extra_kl_Alpha + log_pmu_Beta + extra_kl_Beta
             + logpx_z + kl_structure + kl_semantic)
    return np.float32(total)


import concourse.mybir as mybir  # noqa: E402
